# revision 36
# baseline (speedup 1.0000x reference)
"""DiffAttn Trainium2 kernel: 8-core two-phase implementation.

Phase 1 (8 cores, data-parallel over (batch, seq-quarter)): QKV projections.
  Each core projects 1024 rows of x (host-transposed to [d, t] and staged
  bf16) through bf16 wq/wk/wv, producing qT/kT/vT [128, 1024] f32 (head dim
  on partitions, fp32 PSUM accumulation over d).

Host glue between launches: gathers per-batch kT [128, 4096] f32 and v
  (repacked to per-128-row-chunk layout, bf16), slices per-core q blocks,
  builds the additive causal masks.

Phase 2 (8 cores, causal-balanced slot schedule): differential attention.
  Each core handles 8 q-blocks of 128 rows, one from each "context class"
  k=1..8 (ctx <= 512k), chosen so every core runs an IDENTICAL instruction
  stream (SPMD) with equal work: per class k the core of batch beta with
  in-batch index c4 takes global q-block 4*(k-1) + (c4+k)%4.

  Per slot (processed in descending k so the kernel tail is the smallest
  slot): scores for both head-halves via fp32r matmuls (q-half stationary
  [64,128], kT moving, two 512-chunks paired into one [128,1024] PSUM tile),
  additive -1e4 mask on the diagonal chunk, Exp on ScalarE with fused
  *SCALE and accum_out row-sums, e1/e2 stored bf16. The differential
  softmax is algebraically fused: with r = -lambda*s1/s2,
      w = e1 + r*e2   (DVE, bf16)
      u = w @ v       (per-128-chunk wT via one block-transposing DMA
                       [DmaTransposeAnt], then bf16 matmuls into PSUM)
      out = (1-li) * rmsnorm(u/s1) * ln_w
  which equals rmsnorm((a1 - lambda*a2) @ v) exactly (the 1/s1 factor
  cancels in the rms ratio and is reapplied before the eps-carrying rms).
"""

import math

import numpy as np

import concourse.bass as bass
import concourse.bacc as bacc
import concourse.tile as tile
from concourse import mybir  # noqa: F401

F32 = mybir.dt.float32
F32R = mybir.dt.float32r
BF16 = mybir.dt.bfloat16

B, T, D, HD = 2, 4096, 2048, 128
H = HD // 2
N_CORES = 8
CHUNK = T // 4  # phase-1 rows per core
DT = D // 128  # d-tiles
DEPTH = 12
LAMBDA_INIT = 0.8 - 0.6 * math.exp(-0.3 * DEPTH)
EPS = 1e-5
SCALE = 1.0 / math.sqrt(HD / 2)
NEG = -10000.0
NSLOT = 8
WOFF = [512 * (s * (s + 1) // 2) for s in range(NSLOT)]
WTOT = 512 * sum(range(1, NSLOT + 1))  # 18432


def _build_phase1():
    nc = bacc.Bacc("TRN2", target_bir_lowering=False, debug=False,
                   num_devices=N_CORES)
    xt = nc.dram_tensor("xt", [D, CHUNK], BF16, kind="ExternalInput").ap()
    ws = {
        n: nc.dram_tensor(n, [128, DT * HD], BF16, kind="ExternalInput").ap()
        for n in ("wq", "wk", "wv")
    }
    outs = {
        n: nc.dram_tensor(n, [HD, CHUNK], F32, kind="ExternalOutput").ap()
        for n in ("qt", "kt", "vt")
    }

    with tile.TileContext(nc) as tc:
        with tc.tile_pool(name="wp", bufs=1) as wp, \
             tc.tile_pool(name="xp", bufs=3) as xp, \
             tc.tile_pool(name="op", bufs=2) as op, \
             tc.tile_pool(name="pp", bufs=1, space="PSUM") as pp:
            wtiles = {}
            for n in ("wq", "wk", "wv"):
                wt = wp.tile([128, DT, HD], BF16, tag=f"w_{n}")
                nc.sync.dma_start(
                    out=wt, in_=ws[n].rearrange("p (dt h) -> p dt h", dt=DT))
                wtiles[n] = wt

            # PE warmup: keep the PE busy while the first loads land so the
            # HAM clock gate is released (2.4 GHz) before the real matmuls.
            wu = wp.tile([128, 512], BF16, tag="wu")
            nc.vector.memset(wu, 0.0)
            wu_ps = pp.tile([128, 512], F32, tag="wu_ps")
            for _ in range(8):
                nc.tensor.matmul(wu_ps, lhsT=wu[:, 0:128], rhs=wu,
                                 start=True, stop=True)

            xt_tiles = {}
            for th in range(2):
                accs = {}
                for pi, n in enumerate(("wq", "wk", "wv")):
                    accs[n] = pp.tile([128, 512], F32,
                                      name=f"acc{pi}_{th}",
                                      tag=f"acc{pi}_{th % 2}")
                for di in range(DT):
                    if th == 0:
                        xt_t = xp.tile([128, CHUNK], BF16,
                                       name=f"xt{di}", tag=f"xt{di}", bufs=1)
                        nc.sync.dma_start(
                            out=xt_t, in_=xt[di * 128:(di + 1) * 128, :])
                        xt_tiles[di] = xt_t
                    for n in ("wq", "wk", "wv"):
                        nc.tensor.matmul(
                            accs[n],
                            lhsT=wtiles[n][:, di, :],
                            rhs=xt_tiles[di][:, th * 512:(th + 1) * 512],
                            start=(di == 0),
                            stop=(di == DT - 1),
                        )
                for i, (n, on) in enumerate(
                        (("wq", "qt"), ("wk", "kt"), ("wv", "vt"))):
                    ot = op.tile([128, 512], F32, tag=f"o_{n}")
                    if i % 2 == 0:
                        nc.scalar.copy(ot, accs[n])
                    else:
                        nc.vector.tensor_copy(ot, accs[n])
                    nc.sync.dma_start(
                        out=outs[on][:, th * 512:(th + 1) * 512], in_=ot)

    nc.compile()
    return nc


def _build_phase2():
    nc = bacc.Bacc("TRN2", target_bir_lowering=False, debug=False,
                   num_devices=N_CORES)
    qt = nc.dram_tensor("qt", [HD, NSLOT * 128], F32R, kind="ExternalInput").ap()
    kt = nc.dram_tensor("kt", [HD, T], F32R, kind="ExternalInput").ap()
    vb = nc.dram_tensor("vb", [128, (T // 128) * HD], BF16, kind="ExternalInput").ap()
    msk = nc.dram_tensor("msk", [128, NSLOT * 512], F32, kind="ExternalInput").ap()
    lam = nc.dram_tensor("lam", [4, H], F32, kind="ExternalInput").ap()
    lnw = nc.dram_tensor("lnw", [HD], F32, kind="ExternalInput").ap()
    o = nc.dram_tensor("o", [NSLOT, 128, HD], F32, kind="ExternalOutput").ap()

    with tile.TileContext(nc) as tc:
        with tc.tile_pool(name="cp", bufs=1) as cp, \
             tc.tile_pool(name="sp", bufs=2) as sp, \
             tc.tile_pool(name="ep", bufs=2) as ep, \
             tc.tile_pool(name="wtp", bufs=2) as wtp, \
             tc.tile_pool(name="yp", bufs=1) as yp, \
             tc.tile_pool(name="obp", bufs=2) as obp, \
             tc.tile_pool(name="pps", bufs=3, space="PSUM") as pps, \
             tc.tile_pool(name="ppu", bufs=2, space="PSUM") as ppu, \
             tc.tile_pool(name="drp", bufs=1, space="DRAM") as drp:

            # ---- constant loads ----
            # qt/kt are declared float32r in DRAM (same f32 bits; the PE
            # rounds in the datapath) so they DMA straight into place.
            # kt stays chunked so the first scores start early.
            ktf = cp.tile([128, T], F32R, tag="ktf")
            qtf = cp.tile([128, NSLOT * 128], F32R, tag="qtf")
            nc.sync.dma_start(out=qtf, in_=qt)
            for c in range(T // 512):
                nc.sync.dma_start(out=ktf[:, c * 512:(c + 1) * 512],
                                  in_=kt[:, c * 512:(c + 1) * 512])
            kt1 = ktf[0:64, :]
            kt2 = ktf[64:128, :]
            qt1 = qtf[0:64, :]
            qt2 = qtf[64:128, :]
            vbs = cp.tile([128, T // 128, HD], BF16, tag="vbs")
            nc.sync.dma_start(
                out=vbs, in_=vb.rearrange("p (c h) -> p c h", h=HD))
            msks = cp.tile([128, NSLOT, 512], F32, tag="msks")
            nc.sync.dma_start(
                out=msks, in_=msk.rearrange("p (s f) -> p s f", s=NSLOT))
            lnwb = cp.tile([128, HD], F32, tag="lnwb")
            nc.sync.dma_start(
                out=lnwb,
                in_=bass.AP(tensor=lnw.tensor, offset=lnw.offset,
                            ap=[[0, 128], [1, HD]]))
            nc.vector.tensor_scalar_mul(lnwb, lnwb, 1.0 - LAMBDA_INIT)

            lams = cp.tile([1, 4 * H], F32, tag="lams")
            nc.sync.dma_start(
                out=lams,
                in_=bass.AP(tensor=lam.tensor, offset=lam.offset,
                            ap=[[0, 1], [1, 4 * H]]))

            # PE warmup: release the HAM clock gate during the prologue
            wu = cp.tile([128, 512], BF16, tag="wu")
            nc.vector.memset(wu, 0.0)
            wu_ps = pps.tile([128, 1024], F32, name="wu_ps", tag="s1p",
                             bufs=2)
            for _ in range(8):
                nc.tensor.matmul(wu_ps[:, 0:512], lhsT=wu[:, 0:128], rhs=wu,
                                 start=True, stop=True)

            # ---- lambda scalar ----
            t64 = cp.tile([1, H], F32, tag="t64")
            d1 = cp.tile([1, 1], F32, tag="d1")
            d2 = cp.tile([1, 1], F32, tag="d2")
            nc.vector.tensor_mul(t64, lams[:, 0:H], lams[:, 2 * H:3 * H])
            nc.vector.reduce_sum(d1, t64, axis=mybir.AxisListType.X)
            nc.vector.tensor_mul(t64, lams[:, H:2 * H], lams[:, 3 * H:4 * H])
            nc.vector.reduce_sum(d2, t64, axis=mybir.AxisListType.X)
            nc.scalar.activation(d1, d1, mybir.ActivationFunctionType.Exp)
            nc.scalar.activation(d2, d2, mybir.ActivationFunctionType.Exp)
            lmb = cp.tile([1, 1], F32, tag="lmb")
            nc.vector.tensor_sub(lmb, d1, d2)
            nc.vector.tensor_scalar_add(lmb, lmb, LAMBDA_INIT)
            nc.vector.tensor_scalar_mul(lmb, lmb, -1.0)
            # broadcast to [128, 1] via DRAM round-trip
            lnd = drp.tile([1, 1], F32, tag="lnd")
            nc.sync.dma_start(out=lnd, in_=lmb)
            lamneg = cp.tile([128, 1], F32, tag="lamneg")
            nc.sync.dma_start(
                out=lamneg,
                in_=bass.AP(tensor=lnd.tensor, offset=lnd.offset,
                            ap=[[0, 128], [1, 1]]))

            # ---- slot phase: scores + exp + combine ----
            s1all = cp.tile([128, NSLOT], F32, tag="s1all")
            w_sb = cp.tile([128, WTOT], BF16, tag="w_sb")
            for s in reversed(range(NSLOT)):
                k = s + 1
                e1 = ep.tile([128, T], BF16, tag="e1")
                e2 = ep.tile([128, T], BF16, tag="e2")
                ps1 = sp.tile([128, 4], F32, tag="ps1")
                ps2 = sp.tile([128, 4], F32, tag="ps2")
                npair = (k + 1) // 2
                for ci in range(npair):
                    c0 = 2 * ci
                    has2 = (c0 + 1) < k
                    width = 1024 if has2 else 512
                    s1p = pps.tile([128, 1024], F32, tag="s1p", bufs=2)
                    s2p = pps.tile([128, 1024], F32, tag="s2p", bufs=1)
                    nc.tensor.matmul(
                        s1p[:, 0:512],
                        lhsT=qt1[:, s * 128:(s + 1) * 128],
                        rhs=kt1[:, c0 * 512:(c0 + 1) * 512])
                    nc.tensor.matmul(
                        s2p[:, 0:512],
                        lhsT=qt2[:, s * 128:(s + 1) * 128],
                        rhs=kt2[:, c0 * 512:(c0 + 1) * 512])
                    if has2:
                        nc.tensor.matmul(
                            s1p[:, 512:1024],
                            lhsT=qt1[:, s * 128:(s + 1) * 128],
                            rhs=kt1[:, (c0 + 1) * 512:(c0 + 2) * 512])
                        nc.tensor.matmul(
                            s2p[:, 512:1024],
                            lhsT=qt2[:, s * 128:(s + 1) * 128],
                            rhs=kt2[:, (c0 + 1) * 512:(c0 + 2) * 512])
                    if ci == npair - 1:
                        moff = (k - 1 - c0) * 512
                        nc.vector.tensor_add(s1p[:, moff:moff + 512],
                                             s1p[:, moff:moff + 512],
                                             msks[:, s, :])
                        nc.vector.tensor_add(s2p[:, moff:moff + 512],
                                             s2p[:, moff:moff + 512],
                                             msks[:, s, :])
                    nc.scalar.activation(
                        e1[:, c0 * 512:c0 * 512 + width], s1p[:, 0:width],
                        mybir.ActivationFunctionType.Exp, scale=SCALE,
                        accum_out=ps1[:, ci:ci + 1])
                    nc.scalar.activation(
                        e2[:, c0 * 512:c0 * 512 + width], s2p[:, 0:width],
                        mybir.ActivationFunctionType.Exp, scale=SCALE,
                        accum_out=ps2[:, ci:ci + 1])
                s2t = sp.tile([128, 1], F32, tag="s2t")
                rc2 = sp.tile([128, 1], F32, tag="rc2")
                r = sp.tile([128, 1], F32, tag="r")
                nc.vector.reduce_sum(s1all[:, s:s + 1], ps1[:, 0:npair],
                                     axis=mybir.AxisListType.X)
                nc.vector.reduce_sum(s2t, ps2[:, 0:npair],
                                     axis=mybir.AxisListType.X)
                nc.vector.reciprocal(rc2, s2t)
                nc.vector.tensor_mul(r, s1all[:, s:s + 1], rc2)
                nc.vector.tensor_mul(r, r, lamneg)
                # w = e1 + r * e2  (bf16 ops run in DVE 2x/4x modes)
                nc.vector.tensor_scalar_mul(e2[:, 0:512 * k], e2[:, 0:512 * k], r)
                nc.vector.tensor_tensor(
                    out=w_sb[:, WOFF[s]:WOFF[s] + 512 * k],
                    in0=e1[:, 0:512 * k],
                    in1=e2[:, 0:512 * k],
                    op=mybir.AluOpType.add)

            # ---- AV phase: transpose w chunks and accumulate u ----
            c1a = cp.tile([128, NSLOT], F32, tag="c1a")
            ssall = cp.tile([128, NSLOT], F32, tag="ssall")
            ys = [None] * NSLOT
            for s in reversed(range(NSLOT)):
                k = s + 1
                u = ppu.tile([128, HD], F32, name=f"u{s}", tag="u")
                wt3 = wtp.tile([128, 4 * NSLOT, 128], BF16, name=f"wt3_{s}",
                               tag="wt3")
                nc.sync.dma_start_transpose(
                    out=wt3[:, 0:4 * k, :],
                    in_=w_sb[:, WOFF[s]:WOFF[s] + 512 * k])
                for t in range(4 * k):
                    nc.tensor.matmul(
                        u,
                        lhsT=wt3[:, t, :],
                        rhs=vbs[:, t, :],
                        start=(t == 0),
                        stop=(t == 4 * k - 1))
                # per-slot: y = u * (1/s1)  (also evacuates u PSUM)
                nc.vector.reciprocal(c1a[:, s:s + 1], s1all[:, s:s + 1])
                y = yp.tile([128, HD], F32, tag=f"y{s}")
                nc.vector.tensor_scalar_mul(y, u, c1a[:, s:s + 1])
                scratch = sp.tile([128, HD], F32, tag="sq")
                nc.scalar.activation(scratch, y,
                                     mybir.ActivationFunctionType.Square,
                                     accum_out=ssall[:, s:s + 1])
                ys[s] = y

            # ---- tail: rms denom, scale, store ----
            epsb = cp.tile([128, 1], F32, tag="epsb")
            nc.vector.memset(epsb, EPS)
            den = cp.tile([128, NSLOT], F32, tag="den")
            nc.scalar.activation(den, ssall,
                                 mybir.ActivationFunctionType.Sqrt,
                                 scale=1.0 / HD, bias=epsb)
            rr = cp.tile([128, NSLOT], F32, tag="rr")
            nc.vector.reciprocal(rr, den)
            oall = obp.tile([128, NSLOT, HD], F32, tag="oall")
            for s in range(NSLOT):
                # o = (y * rr[s]) * lnw, fused in one DVE op
                nc.vector.scalar_tensor_tensor(
                    out=oall[:, s, :], in0=ys[s], scalar=rr[:, s:s + 1],
                    in1=lnwb, op0=mybir.AluOpType.mult,
                    op1=mybir.AluOpType.mult)
            nc.sync.dma_start(
                out=o.rearrange("s p h -> p s h"), in_=oall)

    nc.compile()
    return nc


_cache = {}


def _make_runner(nc):
    """Build a persistent jitted SPMD executor for a compiled Bacc program.

    Mirrors concourse.bass2jax.run_bass_via_pjrt but caches the jitted
    callable so repeated kernel() calls don't re-trace/re-compile.
    """
    import jax
    from jax.sharding import Mesh, PartitionSpec
    from jax.experimental.shard_map import shard_map
    from concourse import bass2jax

    bass2jax.install_neuronx_cc_hook()

    partition_name = (nc.partition_id_tensor.name
                      if nc.partition_id_tensor else None)
    in_names, out_names, out_avals = [], [], []
    for alloc in nc.m.functions[0].allocations:
        if not isinstance(alloc, mybir.MemoryLocationSet):
            continue
        name = alloc.memorylocations[0].name
        if alloc.kind == "ExternalInput":
            if name != partition_name:
                in_names.append(name)
        elif alloc.kind == "ExternalOutput":
            out_names.append(name)
            out_avals.append(jax.core.ShapedArray(
                tuple(alloc.tensor_shape), mybir.dt.np(alloc.dtype)))
    n_params = len(in_names)
    all_in_names = list(in_names) + list(out_names)
    if partition_name is not None:
        all_in_names.append(partition_name)
    donate = tuple(range(n_params, n_params + len(out_names)))

    def _body(*args):
        operands = list(args)
        if partition_name is not None:
            operands.append(bass2jax.partition_id_tensor())
        outs = bass2jax._bass_exec_p.bind(
            *operands,
            out_avals=tuple(out_avals),
            in_names=tuple(all_in_names),
            out_names=tuple(out_names),
            lowering_input_output_aliases=(),
            sim_require_finite=True,
            sim_require_nnan=True,
            nc=nc,
        )
        return tuple(outs)

    devices = jax.devices()[:N_CORES]
    mesh = Mesh(np.asarray(devices), ("core",))
    in_specs = (PartitionSpec("core"),) * (n_params + len(out_names))
    out_specs = (PartitionSpec("core"),) * len(out_names)
    sharded = jax.jit(
        shard_map(_body, mesh=mesh, in_specs=in_specs, out_specs=out_specs,
                  check_rep=False),
        donate_argnums=donate, keep_unused=True)

    def run(in_maps):
        concat_in = [
            np.concatenate([np.asarray(in_maps[c][name])
                            for c in range(N_CORES)], axis=0)
            for name in in_names
        ]
        concat_zeros = [
            np.zeros((N_CORES * av.shape[0], *av.shape[1:]), av.dtype)
            for av in out_avals
        ]
        out_arrs = sharded(*concat_in, *concat_zeros)
        return [
            {name: np.asarray(out_arrs[i]).reshape(
                N_CORES, *out_avals[i].shape)[c]
             for i, name in enumerate(out_names)}
            for c in range(N_CORES)
        ]

    return run


def _get_programs():
    if "p1" not in _cache:
        _cache["p1"] = _build_phase1()
        _cache["run1"] = _make_runner(_cache["p1"])
    if "p2" not in _cache:
        _cache["p2"] = _build_phase2()
        _cache["run2"] = _make_runner(_cache["p2"])
    return _cache["p1"], _cache["p2"]


def kernel(x, wq, wk, wv, lambda_q1, lambda_q2, lambda_k1, lambda_k2,
           ln_weight):
    import ml_dtypes

    x = np.asarray(x, np.float32)
    wq = np.asarray(wq, np.float32)
    wk = np.asarray(wk, np.float32)
    wv = np.asarray(wv, np.float32)
    def _pack_w(w):
        # [D, HD] -> [128, DT*HD] with row p, block dt = w[dt*128+p, :]
        return np.ascontiguousarray(
            w.reshape(DT, 128, HD).transpose(1, 0, 2).reshape(128, DT * HD)
        ).astype(ml_dtypes.bfloat16)
    wq_b = _pack_w(wq)
    wk_b = _pack_w(wk)
    wv_b = _pack_w(wv)
    lam = np.stack([
        np.asarray(lambda_q1, np.float32),
        np.asarray(lambda_q2, np.float32),
        np.asarray(lambda_k1, np.float32),
        np.asarray(lambda_k2, np.float32),
    ])
    lnw = np.asarray(ln_weight, np.float32)

    nc1, nc2 = _get_programs()

    # ---- phase 1 ----
    in1 = []
    for c in range(N_CORES):
        beta, c4 = divmod(c, 4)
        xt = np.ascontiguousarray(
            x[beta, c4 * CHUNK:(c4 + 1) * CHUNK, :].T).astype(
                ml_dtypes.bfloat16)
        in1.append({"xt": xt, "wq": wq_b, "wk": wk_b, "wv": wv_b})
    r1 = _cache["run1"](in1)

    qtf, ktf, vf = {}, {}, {}
    for beta in range(B):
        qtf[beta] = np.concatenate(
            [r1[beta * 4 + j]["qt"] for j in range(4)], axis=1)
        ktf[beta] = np.ascontiguousarray(np.concatenate(
            [r1[beta * 4 + j]["kt"] for j in range(4)], axis=1))
        vt = np.concatenate(
            [r1[beta * 4 + j]["vt"] for j in range(4)], axis=1)
        v = vt.T  # [T, HD]
        vf[beta] = np.ascontiguousarray(
            v.reshape(T // 128, 128, HD).transpose(1, 0, 2).reshape(
                128, (T // 128) * HD)).astype(ml_dtypes.bfloat16)

    # ---- phase 2 ----
    p_idx = np.arange(128)[:, None]
    f_idx = np.arange(512)[None, :]
    in2 = []
    for c in range(N_CORES):
        beta, c4 = divmod(c, 4)
        qsl = np.empty((HD, NSLOT * 128), np.float32)
        mk = np.empty((NSLOT, 128, 512), np.float32)
        for s in range(NSLOT):
            t = (c4 + s + 1) % 4
            g = 4 * s + t
            qsl[:, s * 128:(s + 1) * 128] = qtf[beta][:, g * 128:(g + 1) * 128]
            mk[s] = np.where(f_idx - p_idx <= 128 * t, 0.0, NEG)
        mk = np.ascontiguousarray(mk.transpose(1, 0, 2).reshape(
            128, NSLOT * 512))
        in2.append({
            "qt": np.ascontiguousarray(qsl),
            "kt": ktf[beta],
            "vb": vf[beta],
            "msk": mk,
            "lam": lam,
            "lnw": lnw,
        })
    r2 = _cache["run2"](in2)

    out = np.empty((B, T, HD), np.float32)
    for c in range(N_CORES):
        beta, c4 = divmod(c, 4)
        for s in range(NSLOT):
            t = (c4 + s + 1) % 4
            g = 4 * s + t
            out[beta, g * 128:(g + 1) * 128, :] = r2[c]["o"][s]
    return out


# revision 39
# speedup vs baseline: 1.0083x; 1.0083x over previous
"""DiffAttn Trainium2 kernel: 8-core two-phase implementation.

Phase 1 (8 cores, data-parallel over (batch, seq-quarter)): QKV projections.
  Each core projects 1024 rows of x (host-transposed to [d, t] and staged
  bf16) through bf16 wq/wk/wv, producing qT/kT/vT [128, 1024] f32 (head dim
  on partitions, fp32 PSUM accumulation over d).

Host glue between launches: gathers per-batch kT [128, 4096] f32 and v
  (repacked to per-128-row-chunk layout, bf16), slices per-core q blocks,
  builds the additive causal masks.

Phase 2 (8 cores, causal-balanced slot schedule): differential attention.
  Each core handles 8 q-blocks of 128 rows, one from each "context class"
  k=1..8 (ctx <= 512k), chosen so every core runs an IDENTICAL instruction
  stream (SPMD) with equal work: per class k the core of batch beta with
  in-batch index c4 takes global q-block 4*(k-1) + (c4+k)%4.

  Per slot (processed in descending k so the kernel tail is the smallest
  slot): scores for both head-halves via fp32r matmuls (q-half stationary
  [64,128], kT moving, two 512-chunks paired into one [128,1024] PSUM tile),
  additive -1e4 mask on the diagonal chunk, Exp on ScalarE with fused
  *SCALE and accum_out row-sums, e1/e2 stored bf16. The differential
  softmax is algebraically fused: with r = -lambda*s1/s2,
      w = e1 + r*e2   (DVE, bf16)
      u = w @ v       (per-128-chunk wT via one block-transposing DMA
                       [DmaTransposeAnt], then bf16 matmuls into PSUM)
      out = (1-li) * rmsnorm(u/s1) * ln_w
  which equals rmsnorm((a1 - lambda*a2) @ v) exactly (the 1/s1 factor
  cancels in the rms ratio and is reapplied before the eps-carrying rms).
"""

import math

import numpy as np

import concourse.bass as bass
import concourse.bacc as bacc
import concourse.tile as tile
from concourse import mybir  # noqa: F401

F32 = mybir.dt.float32
F32R = mybir.dt.float32r
BF16 = mybir.dt.bfloat16

B, T, D, HD = 2, 4096, 2048, 128
H = HD // 2
N_CORES = 8
CHUNK = T // 4  # phase-1 rows per core
DT = D // 128  # d-tiles
DEPTH = 12
LAMBDA_INIT = 0.8 - 0.6 * math.exp(-0.3 * DEPTH)
EPS = 1e-5
SCALE = 1.0 / math.sqrt(HD / 2)
NEG = -10000.0
NSLOT = 8
WOFF = [512 * (s * (s + 1) // 2) for s in range(NSLOT)]
WTOT = 512 * sum(range(1, NSLOT + 1))  # 18432


def _build_phase1():
    nc = bacc.Bacc("TRN2", target_bir_lowering=False, debug=False,
                   num_devices=N_CORES)
    xt = nc.dram_tensor("xt", [D, CHUNK], BF16, kind="ExternalInput").ap()
    ws = {
        n: nc.dram_tensor(n, [128, DT * HD], BF16, kind="ExternalInput").ap()
        for n in ("wq", "wk", "wv")
    }
    outs = {
        n: nc.dram_tensor(n, [HD, CHUNK], F32, kind="ExternalOutput").ap()
        for n in ("qt", "kt", "vt")
    }

    with tile.TileContext(nc) as tc:
        with tc.tile_pool(name="wp", bufs=1) as wp, \
             tc.tile_pool(name="xp", bufs=3) as xp, \
             tc.tile_pool(name="op", bufs=2) as op, \
             tc.tile_pool(name="pp", bufs=1, space="PSUM") as pp:
            wtiles = {}
            for n in ("wq", "wk", "wv"):
                wt = wp.tile([128, DT, HD], BF16, tag=f"w_{n}")
                nc.sync.dma_start(
                    out=wt, in_=ws[n].rearrange("p (dt h) -> p dt h", dt=DT))
                wtiles[n] = wt

            # PE warmup: keep the PE busy while the first loads land so the
            # HAM clock gate is released (2.4 GHz) before the real matmuls.
            wu = wp.tile([128, 512], BF16, tag="wu")
            nc.vector.memset(wu, 0.0)
            wu_ps = pp.tile([128, 512], F32, tag="wu_ps")
            for _ in range(8):
                nc.tensor.matmul(wu_ps, lhsT=wu[:, 0:128], rhs=wu,
                                 start=True, stop=True)

            xt_tiles = {}
            for th in range(2):
                accs = {}
                for pi, n in enumerate(("wq", "wk", "wv")):
                    accs[n] = pp.tile([128, 512], F32,
                                      name=f"acc{pi}_{th}",
                                      tag=f"acc{pi}_{th % 2}")
                for di in range(DT):
                    if th == 0:
                        xt_t = xp.tile([128, CHUNK], BF16,
                                       name=f"xt{di}", tag=f"xt{di}", bufs=1)
                        nc.sync.dma_start(
                            out=xt_t, in_=xt[di * 128:(di + 1) * 128, :])
                        xt_tiles[di] = xt_t
                    for n in ("wq", "wk", "wv"):
                        nc.tensor.matmul(
                            accs[n],
                            lhsT=wtiles[n][:, di, :],
                            rhs=xt_tiles[di][:, th * 512:(th + 1) * 512],
                            start=(di == 0),
                            stop=(di == DT - 1),
                        )
                for i, (n, on) in enumerate(
                        (("wq", "qt"), ("wk", "kt"), ("wv", "vt"))):
                    ot = op.tile([128, 512], F32, tag=f"o_{n}")
                    if i % 2 == 0:
                        nc.scalar.copy(ot, accs[n])
                    else:
                        nc.vector.tensor_copy(ot, accs[n])
                    nc.sync.dma_start(
                        out=outs[on][:, th * 512:(th + 1) * 512], in_=ot)

    nc.compile()
    return nc


def _build_phase2():
    nc = bacc.Bacc("TRN2", target_bir_lowering=False, debug=False,
                   num_devices=N_CORES)
    qt = nc.dram_tensor("qt", [HD, NSLOT * 128], F32R, kind="ExternalInput").ap()
    kt = nc.dram_tensor("kt", [HD, T], F32R, kind="ExternalInput").ap()
    vb = nc.dram_tensor("vb", [128, (T // 128) * HD], BF16, kind="ExternalInput").ap()
    msk = nc.dram_tensor("msk", [128, NSLOT * 512], F32, kind="ExternalInput").ap()
    lam = nc.dram_tensor("lam", [4, H], F32, kind="ExternalInput").ap()
    lnw = nc.dram_tensor("lnw", [HD], F32, kind="ExternalInput").ap()
    o = nc.dram_tensor("o", [NSLOT, 128, HD], F32, kind="ExternalOutput").ap()

    with tile.TileContext(nc) as tc:
        with tc.tile_pool(name="cp", bufs=1) as cp, \
             tc.tile_pool(name="sp", bufs=2) as sp, \
             tc.tile_pool(name="ep", bufs=2) as ep, \
             tc.tile_pool(name="wtp", bufs=2) as wtp, \
             tc.tile_pool(name="yp", bufs=1) as yp, \
             tc.tile_pool(name="obp", bufs=2) as obp, \
             tc.tile_pool(name="pps", bufs=3, space="PSUM") as pps, \
             tc.tile_pool(name="ppu", bufs=2, space="PSUM") as ppu, \
             tc.tile_pool(name="drp", bufs=1, space="DRAM") as drp:

            # ---- constant loads ----
            # qt/kt are declared float32r in DRAM (same f32 bits; the PE
            # rounds in the datapath) so they DMA straight into place.
            # kt stays chunked so the first scores start early.
            ktf = cp.tile([128, T], F32R, tag="ktf")
            qtf = cp.tile([128, NSLOT * 128], F32R, tag="qtf")
            nc.sync.dma_start(out=qtf, in_=qt)
            for c in range(T // 512):
                nc.sync.dma_start(out=ktf[:, c * 512:(c + 1) * 512],
                                  in_=kt[:, c * 512:(c + 1) * 512])
            kt1 = ktf[0:64, :]
            kt2 = ktf[64:128, :]
            qt1 = qtf[0:64, :]
            qt2 = qtf[64:128, :]
            vbs = cp.tile([128, T // 128, HD], BF16, tag="vbs")
            nc.sync.dma_start(
                out=vbs, in_=vb.rearrange("p (c h) -> p c h", h=HD))
            msks = cp.tile([128, NSLOT, 512], F32, tag="msks")
            nc.sync.dma_start(
                out=msks, in_=msk.rearrange("p (s f) -> p s f", s=NSLOT))
            lnwb = cp.tile([128, HD], F32, tag="lnwb")
            nc.sync.dma_start(
                out=lnwb,
                in_=bass.AP(tensor=lnw.tensor, offset=lnw.offset,
                            ap=[[0, 128], [1, HD]]))
            nc.vector.tensor_scalar_mul(lnwb, lnwb, 1.0 - LAMBDA_INIT)

            lams = cp.tile([1, 4 * H], F32, tag="lams")
            nc.sync.dma_start(
                out=lams,
                in_=bass.AP(tensor=lam.tensor, offset=lam.offset,
                            ap=[[0, 1], [1, 4 * H]]))

            # PE warmup: release the HAM clock gate during the prologue
            wu = cp.tile([128, 512], BF16, tag="wu")
            nc.vector.memset(wu, 0.0)
            wu_ps = pps.tile([128, 1024], F32, name="wu_ps", tag="sp",
                             bufs=3)
            for _ in range(8):
                nc.tensor.matmul(wu_ps[:, 0:512], lhsT=wu[:, 0:128], rhs=wu,
                                 start=True, stop=True)

            # ---- lambda scalar ----
            t64 = cp.tile([1, H], F32, tag="t64")
            d1 = cp.tile([1, 1], F32, tag="d1")
            d2 = cp.tile([1, 1], F32, tag="d2")
            nc.vector.tensor_mul(t64, lams[:, 0:H], lams[:, 2 * H:3 * H])
            nc.vector.reduce_sum(d1, t64, axis=mybir.AxisListType.X)
            nc.vector.tensor_mul(t64, lams[:, H:2 * H], lams[:, 3 * H:4 * H])
            nc.vector.reduce_sum(d2, t64, axis=mybir.AxisListType.X)
            nc.scalar.activation(d1, d1, mybir.ActivationFunctionType.Exp)
            nc.scalar.activation(d2, d2, mybir.ActivationFunctionType.Exp)
            lmb = cp.tile([1, 1], F32, tag="lmb")
            nc.vector.tensor_sub(lmb, d1, d2)
            nc.vector.tensor_scalar_add(lmb, lmb, LAMBDA_INIT)
            nc.vector.tensor_scalar_mul(lmb, lmb, -1.0)
            # broadcast to [128, 1] via DRAM round-trip
            lnd = drp.tile([1, 1], F32, tag="lnd")
            nc.sync.dma_start(out=lnd, in_=lmb)
            lamneg = cp.tile([128, 1], F32, tag="lamneg")
            nc.sync.dma_start(
                out=lamneg,
                in_=bass.AP(tensor=lnd.tensor, offset=lnd.offset,
                            ap=[[0, 128], [1, 1]]))

            # ---- slot phase: scores + exp + combine ----
            s1all = cp.tile([128, NSLOT], F32, tag="s1all")
            w_sb = cp.tile([128, WTOT], BF16, tag="w_sb")
            for s in reversed(range(NSLOT)):
                k = s + 1
                e1 = ep.tile([128, T], BF16, tag="e1")
                e2 = ep.tile([128, T], BF16, tag="e2")
                ps1 = sp.tile([128, 4], F32, tag="ps1")
                ps2 = sp.tile([128, 4], F32, tag="ps2")
                npair = (k + 1) // 2
                for ci in range(npair):
                    c0 = 2 * ci
                    has2 = (c0 + 1) < k
                    width = 1024 if has2 else 512
                    s1p = pps.tile([128, 1024], F32, tag="sp", bufs=3)
                    s2p = pps.tile([128, 1024], F32, tag="sp", bufs=3)
                    nc.tensor.matmul(
                        s1p[:, 0:512],
                        lhsT=qt1[:, s * 128:(s + 1) * 128],
                        rhs=kt1[:, c0 * 512:(c0 + 1) * 512])
                    nc.tensor.matmul(
                        s2p[:, 0:512],
                        lhsT=qt2[:, s * 128:(s + 1) * 128],
                        rhs=kt2[:, c0 * 512:(c0 + 1) * 512])
                    if has2:
                        nc.tensor.matmul(
                            s1p[:, 512:1024],
                            lhsT=qt1[:, s * 128:(s + 1) * 128],
                            rhs=kt1[:, (c0 + 1) * 512:(c0 + 2) * 512])
                        nc.tensor.matmul(
                            s2p[:, 512:1024],
                            lhsT=qt2[:, s * 128:(s + 1) * 128],
                            rhs=kt2[:, (c0 + 1) * 512:(c0 + 2) * 512])
                    if ci == npair - 1:
                        moff = (k - 1 - c0) * 512
                        nc.vector.tensor_add(s1p[:, moff:moff + 512],
                                             s1p[:, moff:moff + 512],
                                             msks[:, s, :])
                        nc.vector.tensor_add(s2p[:, moff:moff + 512],
                                             s2p[:, moff:moff + 512],
                                             msks[:, s, :])
                    nc.scalar.activation(
                        e1[:, c0 * 512:c0 * 512 + width], s1p[:, 0:width],
                        mybir.ActivationFunctionType.Exp, scale=SCALE,
                        accum_out=ps1[:, ci:ci + 1])
                    nc.scalar.activation(
                        e2[:, c0 * 512:c0 * 512 + width], s2p[:, 0:width],
                        mybir.ActivationFunctionType.Exp, scale=SCALE,
                        accum_out=ps2[:, ci:ci + 1])
                s2t = sp.tile([128, 1], F32, tag="s2t")
                rc2 = sp.tile([128, 1], F32, tag="rc2")
                r = sp.tile([128, 1], F32, tag="r")
                nc.vector.reduce_sum(s1all[:, s:s + 1], ps1[:, 0:npair],
                                     axis=mybir.AxisListType.X)
                nc.vector.reduce_sum(s2t, ps2[:, 0:npair],
                                     axis=mybir.AxisListType.X)
                nc.vector.reciprocal(rc2, s2t)
                nc.vector.tensor_mul(r, s1all[:, s:s + 1], rc2)
                nc.vector.tensor_mul(r, r, lamneg)
                # w = e1 + r * e2  (bf16 ops run in DVE 2x/4x modes)
                nc.vector.tensor_scalar_mul(e2[:, 0:512 * k], e2[:, 0:512 * k], r)
                nc.vector.tensor_tensor(
                    out=w_sb[:, WOFF[s]:WOFF[s] + 512 * k],
                    in0=e1[:, 0:512 * k],
                    in1=e2[:, 0:512 * k],
                    op=mybir.AluOpType.add)

            # ---- AV phase: transpose w chunks and accumulate u ----
            c1a = cp.tile([128, NSLOT], F32, tag="c1a")
            ssall = cp.tile([128, NSLOT], F32, tag="ssall")
            ys = [None] * NSLOT
            for s in reversed(range(NSLOT)):
                k = s + 1
                u = ppu.tile([128, HD], F32, name=f"u{s}", tag="u")
                wt3 = wtp.tile([128, 4 * NSLOT, 128], BF16, name=f"wt3_{s}",
                               tag="wt3")
                nc.sync.dma_start_transpose(
                    out=wt3[:, 0:4 * k, :],
                    in_=w_sb[:, WOFF[s]:WOFF[s] + 512 * k])
                for t in range(4 * k):
                    nc.tensor.matmul(
                        u,
                        lhsT=wt3[:, t, :],
                        rhs=vbs[:, t, :],
                        start=(t == 0),
                        stop=(t == 4 * k - 1))
                # per-slot: y = u * (1/s1)  (also evacuates u PSUM)
                nc.vector.reciprocal(c1a[:, s:s + 1], s1all[:, s:s + 1])
                y = yp.tile([128, HD], F32, tag=f"y{s}")
                nc.vector.tensor_scalar_mul(y, u, c1a[:, s:s + 1])
                scratch = sp.tile([128, HD], F32, tag="sq")
                nc.scalar.activation(scratch, y,
                                     mybir.ActivationFunctionType.Square,
                                     accum_out=ssall[:, s:s + 1])
                ys[s] = y

            # ---- tail: rms denom, scale, store ----
            epsb = cp.tile([128, 1], F32, tag="epsb")
            nc.vector.memset(epsb, EPS)
            den = cp.tile([128, NSLOT], F32, tag="den")
            nc.scalar.activation(den, ssall,
                                 mybir.ActivationFunctionType.Sqrt,
                                 scale=1.0 / HD, bias=epsb)
            rr = cp.tile([128, NSLOT], F32, tag="rr")
            nc.vector.reciprocal(rr, den)
            oall = obp.tile([128, NSLOT, HD], F32, tag="oall")
            for s in range(NSLOT):
                # o = (y * rr[s]) * lnw, fused in one DVE op
                nc.vector.scalar_tensor_tensor(
                    out=oall[:, s, :], in0=ys[s], scalar=rr[:, s:s + 1],
                    in1=lnwb, op0=mybir.AluOpType.mult,
                    op1=mybir.AluOpType.mult)
            nc.sync.dma_start(
                out=o.rearrange("s p h -> p s h"), in_=oall)

    nc.compile()
    return nc


_cache = {}


def _make_runner(nc):
    """Build a persistent jitted SPMD executor for a compiled Bacc program.

    Mirrors concourse.bass2jax.run_bass_via_pjrt but caches the jitted
    callable so repeated kernel() calls don't re-trace/re-compile.
    """
    import jax
    from jax.sharding import Mesh, PartitionSpec
    from jax.experimental.shard_map import shard_map
    from concourse import bass2jax

    bass2jax.install_neuronx_cc_hook()

    partition_name = (nc.partition_id_tensor.name
                      if nc.partition_id_tensor else None)
    in_names, out_names, out_avals = [], [], []
    for alloc in nc.m.functions[0].allocations:
        if not isinstance(alloc, mybir.MemoryLocationSet):
            continue
        name = alloc.memorylocations[0].name
        if alloc.kind == "ExternalInput":
            if name != partition_name:
                in_names.append(name)
        elif alloc.kind == "ExternalOutput":
            out_names.append(name)
            out_avals.append(jax.core.ShapedArray(
                tuple(alloc.tensor_shape), mybir.dt.np(alloc.dtype)))
    n_params = len(in_names)
    all_in_names = list(in_names) + list(out_names)
    if partition_name is not None:
        all_in_names.append(partition_name)
    donate = tuple(range(n_params, n_params + len(out_names)))

    def _body(*args):
        operands = list(args)
        if partition_name is not None:
            operands.append(bass2jax.partition_id_tensor())
        outs = bass2jax._bass_exec_p.bind(
            *operands,
            out_avals=tuple(out_avals),
            in_names=tuple(all_in_names),
            out_names=tuple(out_names),
            lowering_input_output_aliases=(),
            sim_require_finite=True,
            sim_require_nnan=True,
            nc=nc,
        )
        return tuple(outs)

    devices = jax.devices()[:N_CORES]
    mesh = Mesh(np.asarray(devices), ("core",))
    in_specs = (PartitionSpec("core"),) * (n_params + len(out_names))
    out_specs = (PartitionSpec("core"),) * len(out_names)
    sharded = jax.jit(
        shard_map(_body, mesh=mesh, in_specs=in_specs, out_specs=out_specs,
                  check_rep=False),
        donate_argnums=donate, keep_unused=True)

    def run(in_maps):
        concat_in = [
            np.concatenate([np.asarray(in_maps[c][name])
                            for c in range(N_CORES)], axis=0)
            for name in in_names
        ]
        concat_zeros = [
            np.zeros((N_CORES * av.shape[0], *av.shape[1:]), av.dtype)
            for av in out_avals
        ]
        out_arrs = sharded(*concat_in, *concat_zeros)
        return [
            {name: np.asarray(out_arrs[i]).reshape(
                N_CORES, *out_avals[i].shape)[c]
             for i, name in enumerate(out_names)}
            for c in range(N_CORES)
        ]

    return run


def _get_programs():
    if "p1" not in _cache:
        _cache["p1"] = _build_phase1()
        _cache["run1"] = _make_runner(_cache["p1"])
    if "p2" not in _cache:
        _cache["p2"] = _build_phase2()
        _cache["run2"] = _make_runner(_cache["p2"])
    return _cache["p1"], _cache["p2"]


def kernel(x, wq, wk, wv, lambda_q1, lambda_q2, lambda_k1, lambda_k2,
           ln_weight):
    import ml_dtypes

    x = np.asarray(x, np.float32)
    wq = np.asarray(wq, np.float32)
    wk = np.asarray(wk, np.float32)
    wv = np.asarray(wv, np.float32)
    def _pack_w(w):
        # [D, HD] -> [128, DT*HD] with row p, block dt = w[dt*128+p, :]
        return np.ascontiguousarray(
            w.reshape(DT, 128, HD).transpose(1, 0, 2).reshape(128, DT * HD)
        ).astype(ml_dtypes.bfloat16)
    wq_b = _pack_w(wq)
    wk_b = _pack_w(wk)
    wv_b = _pack_w(wv)
    lam = np.stack([
        np.asarray(lambda_q1, np.float32),
        np.asarray(lambda_q2, np.float32),
        np.asarray(lambda_k1, np.float32),
        np.asarray(lambda_k2, np.float32),
    ])
    lnw = np.asarray(ln_weight, np.float32)

    nc1, nc2 = _get_programs()

    # ---- phase 1 ----
    in1 = []
    for c in range(N_CORES):
        beta, c4 = divmod(c, 4)
        xt = np.ascontiguousarray(
            x[beta, c4 * CHUNK:(c4 + 1) * CHUNK, :].T).astype(
                ml_dtypes.bfloat16)
        in1.append({"xt": xt, "wq": wq_b, "wk": wk_b, "wv": wv_b})
    r1 = _cache["run1"](in1)

    qtf, ktf, vf = {}, {}, {}
    for beta in range(B):
        qtf[beta] = np.concatenate(
            [r1[beta * 4 + j]["qt"] for j in range(4)], axis=1)
        ktf[beta] = np.ascontiguousarray(np.concatenate(
            [r1[beta * 4 + j]["kt"] for j in range(4)], axis=1))
        vt = np.concatenate(
            [r1[beta * 4 + j]["vt"] for j in range(4)], axis=1)
        v = vt.T  # [T, HD]
        vf[beta] = np.ascontiguousarray(
            v.reshape(T // 128, 128, HD).transpose(1, 0, 2).reshape(
                128, (T // 128) * HD)).astype(ml_dtypes.bfloat16)

    # ---- phase 2 ----
    p_idx = np.arange(128)[:, None]
    f_idx = np.arange(512)[None, :]
    in2 = []
    for c in range(N_CORES):
        beta, c4 = divmod(c, 4)
        qsl = np.empty((HD, NSLOT * 128), np.float32)
        mk = np.empty((NSLOT, 128, 512), np.float32)
        for s in range(NSLOT):
            t = (c4 + s + 1) % 4
            g = 4 * s + t
            qsl[:, s * 128:(s + 1) * 128] = qtf[beta][:, g * 128:(g + 1) * 128]
            mk[s] = np.where(f_idx - p_idx <= 128 * t, 0.0, NEG)
        mk = np.ascontiguousarray(mk.transpose(1, 0, 2).reshape(
            128, NSLOT * 512))
        in2.append({
            "qt": np.ascontiguousarray(qsl),
            "kt": ktf[beta],
            "vb": vf[beta],
            "msk": mk,
            "lam": lam,
            "lnw": lnw,
        })
    r2 = _cache["run2"](in2)

    out = np.empty((B, T, HD), np.float32)
    for c in range(N_CORES):
        beta, c4 = divmod(c, 4)
        for s in range(NSLOT):
            t = (c4 + s + 1) % 4
            g = 4 * s + t
            out[beta, g * 128:(g + 1) * 128, :] = r2[c]["o"][s]
    return out
